# revision 78
# baseline (speedup 1.0000x reference)
"""Trainium2 Bass kernel for causal self-attention with PoPE (v2).

Reference computation (B=2, T=2048, C=1024, H=16, D=64):
  qkv = x @ w_attn.T ; split q,k,v ; heads
  mu_q = softplus(q); mu_k = softplus(k)
  q_real = mu_q * cos(t w); q_imag = mu_q * sin(t w)
  k_real = mu_k * cos(t w + d); k_imag = mu_k * sin(t w + d)
  att = softmax_causal((q_real k_real + q_imag k_imag)/sqrt(D))
  y = att @ v ; out = y @ w_proj.T

Sharding: 8 cores = 2 batches x 4 head-groups (4 heads each). Each core
computes its batch's QKV for its heads, attention, and a partial c_proj
(its heads' input-channel rows of w_proj). Host sums the 4 partials per
batch.

v2 design (132.9us vs the previous 172.7us version; ACT ~110us busy
is the bottleneck engine, PE ~100us):
  * all-bf16 tensors (the NEFF verifier rejects mixed f32r/bf16
    matmuls): halves DMA bytes, keeps every matmul at 1 cycle/row, and
    doubles DVE throughput on the elementwise path.
  * y matmul flipped to out[tq,d] (free dim 65 instead of 512): halves
    its PE cost AND lands the softmax denominator as a per-partition
    scalar (psum col 64), so normalization is a [128,4] reciprocal + 4
    tensor_scalar ops instead of reciprocal+PE-broadcast+mul. The sweep
    runs b-outer so only one psum accumulation group is open per bank
    (ZERO_REGION = a full 2KB bank).
  * y^T for c_proj via PE transpose (2 heads x 1 tq-128 block per op;
    gpsimd cannot read PSUM, so all psum->sbuf copies ride DVE/ACT).
  * c_proj fused per tq-128 block right after the last head's
    transpose: output DMAs spread across the whole attention phase; the
    very last pass runs per-b chains, largest b first, so the kernel
    tail is one short chain.
  * Phase A softplus = exp+ln batched per head-pair with explicit ACT
    ordering (the act-table pass otherwise reloads tables on every
    Exp<->Ln switch: 5 loads instead of 15); diagonal score blocks pair
    with non-diagonal ones so each psum tile needs a single contiguous
    exp.
  * every psum pool is opened once at the top (QK shares the S pool's
    tag) - no pool-boundary chaining stalls at phase transitions.
  * the (j,h) y-sweep is split into stages consumed one per S-pair of
    the following passes (up to 3 carried), so the in-order PE queue
    alternates score matmuls with sweep chunks and ACT never starves.
  * mu partition swaps via SBUF->SBUF DMA on the idle SP queue instead
    of DVE stream_shuffle; xT DMAs split per tbp-half across all three
    DMA-capable queues so the first QK psum completes ~6us in.
"""

import math
import os
import sys

import numpy as np

for _p in ("/opt/trn_rl_repo",):
    if _p not in sys.path and os.path.isdir(_p):
        sys.path.insert(0, _p)

import ml_dtypes

import concourse.tile as tile
from concourse import bacc
from concourse import mybir
from concourse import bass_utils

B, T, C = 2, 2048, 1024
H, D = 16, 64
BASE = 10000.0
N_CORES = 8
HPC = 4  # heads per core
NCT = 8  # c tiles (1024/128)
NTT = 16  # t tiles of 128

F32 = mybir.dt.float32
F32R = mybir.dt.float32r
BF16 = mybir.dt.bfloat16
AF = mybir.ActivationFunctionType


def build_module():
    nc = bacc.Bacc(
        "TRN2", target_bir_lowering=False, debug=False, num_devices=N_CORES
    )

    xT_d = nc.dram_tensor("xT", (NCT, 128, T), BF16, kind="ExternalInput").ap()
    wqk_d = nc.dram_tensor("wqk", (NCT, 128, 512), BF16, kind="ExternalInput").ap()
    wv_d = nc.dram_tensor("wv", (NCT, 128, 256), BF16, kind="ExternalInput").ap()
    w2_d = nc.dram_tensor("w2", (2, 128, 1024), BF16, kind="ExternalInput").ap()
    trig_d = nc.dram_tensor("trig", (128, T), BF16, kind="ExternalInput").ap()
    ab_d = nc.dram_tensor("ab", (HPC, 128, T), BF16, kind="ExternalInput").ap()
    cmask_d = nc.dram_tensor("cmask", (128, 128), BF16, kind="ExternalInput").ap()
    ident_d = nc.dram_tensor("ident", (128, 128), BF16, kind="ExternalInput").ap()
    out_d = nc.dram_tensor("out", (NTT, 128, 1024), F32, kind="ExternalOutput").ap()

    idm = list(range(32))

    with tile.TileContext(nc) as tc:
        with (
            tc.tile_pool(name="persist", bufs=1) as persist,
            tc.tile_pool(name="shufp", bufs=3) as shufp,
            tc.tile_pool(name="pp", bufs=12) as pp,
            tc.tile_pool(name="rcp", bufs=4) as rcp,
            tc.tile_pool(name="ynp", bufs=3) as ynp,
            tc.tile_pool(name="ostp", bufs=8) as ostp,
            tc.tile_pool(name="ps_s", bufs=2, space="PSUM") as ps_s,
            tc.tile_pool(name="ps_y", bufs=1, space="PSUM") as ps_y,
            tc.tile_pool(name="ps_t", bufs=1, space="PSUM") as ps_t,
            tc.tile_pool(name="ps_o", bufs=2, space="PSUM") as ps_o,
        ):
            # ---------------- persistent tiles ----------------
            v_aug = persist.tile([128, NTT, HPC, 65], BF16)
            nc.vector.memset(v_aug[:, :, :, 64:65], 1.0)
            trig = persist.tile([128, T], BF16)
            ab = persist.tile([128, HPC, T], BF16)
            cmask = persist.tile([128, 128], BF16)
            ident = persist.tile([128, 128], BF16)
            bias_m1 = persist.tile([128, 1], F32)
            nc.vector.memset(bias_m1, -1.0)
            mu = [persist.tile([128, T], BF16, name=f"mu{h}") for h in range(HPC)]
            qt = [persist.tile([128, T], BF16, name=f"qt{h}") for h in range(HPC)]
            kt = [persist.tile([128, T], BF16, name=f"kt{h}") for h in range(HPC)]
            y_t = persist.tile([128, 2, T], BF16)
            w2 = persist.tile([128, 2, 1024], BF16)
            wqk = persist.tile([128, NCT, 512], BF16)
            wv = persist.tile([128, NCT, 256], BF16)
            xT = persist.tile([128, NCT, T], BF16)

            # ------------- input DMAs (chunked; ACT only at idle start) ----
            xt_eng = {0: nc.sync, 3: nc.sync, 6: nc.sync,
                      1: nc.gpsimd, 4: nc.gpsimd, 7: nc.gpsimd,
                      2: nc.scalar, 5: nc.scalar}
            for c in range(NCT):
                weng = nc.gpsimd if c >= 6 else nc.sync
                weng.dma_start(wqk[:, c, :], wqk_d[c])
                # tbp-half granularity: the first QK psum only needs the
                # t<1024 halves, so its matmuls start ~2us earlier
                xt_eng[c].dma_start(xT[:, c, 0:1024], xT_d[c][:, 0:1024])
            for c in range(NCT):
                xt_eng[c].dma_start(xT[:, c, 1024:2048], xT_d[c][:, 1024:2048])
            nc.gpsimd.dma_start(wv, wv_d.rearrange("o p e -> p o e"))
            # trig/ab0-1 on the ACT queue: it is idle until the first QK
            # psum completes, so these transfers are free there; ab2-3
            # ride late on SP (needed only at prep h2/h3)
            nc.scalar.dma_start(trig, trig_d)
            for h in range(2):
                nc.scalar.dma_start(ab[:, h, :], ab_d[h])
            nc.gpsimd.dma_start(cmask, cmask_d)
            nc.sync.dma_start(ident, ident_d)
            nc.sync.dma_start(w2, w2_d.rearrange("o p e -> p o e"))
            for h in range(2, HPC):
                nc.sync.dma_start(ab[:, h, :], ab_d[h])

            # ---------------- Phase A: QKV projection ----------------
            # exp/ln batched per head-pair, with explicit ACT ordering so
            # the scheduler can't interleave Exp<->Ln (each switch costs a
            # 1283ns act-table reload): 5 loads total instead of 15.
            from concourse.tile_rust import add_dep_helper

            last_act = [None]

            def ordered_act(handle):
                if last_act[0] is not None:
                    add_dep_helper(
                        handle.ins, last_act[0].ins, sync=False,
                        reason="batch ACT ops to minimize table reloads",
                    )
                last_act[0] = handle

            def v_chunk(tts):
                # V tiles borrow the ps_o pool (temporally disjoint from
                # c_proj use: V at chunk start, c_proj at end)
                for tt in tts:
                    psv = ps_o.tile([128, 512], F32, tag="o", name="ps_o")
                    for c in range(NCT):
                        nc.tensor.matmul(
                            psv[:, 0:256],
                            lhsT=xT[:, c, tt * 128 : (tt + 1) * 128],
                            rhs=wv[:, c, :],
                            start=(c == 0),
                            stop=(c == NCT - 1),
                        )
                    nc.vector.tensor_copy(
                        out=v_aug[:, tt, :, 0:64],
                        in_=psv[:, 0:256].rearrange("p (h e) -> p h e", h=HPC),
                    )

            def phaseA_pair(hp, interleave=()):
                k = 0
                for h in (2 * hp, 2 * hp + 1):
                    for tbp in range(2):
                        ps = ps_s.tile([128, 1024], F32, tag="s", name="ps_s")
                        base = tbp * 1024
                        for c in range(NCT):
                            for half in range(2):
                                o0 = half * 512
                                nc.tensor.matmul(
                                    ps[:, o0 : o0 + 512],
                                    lhsT=wqk[:, c, h * 128 : (h + 1) * 128],
                                    rhs=xT[:, c, base + o0 : base + o0 + 512],
                                    start=(c == 0),
                                    stop=(c == NCT - 1),
                                )
                        ordered_act(
                            nc.scalar.activation(
                                mu[h][:, base : base + 1024], ps, AF.Exp
                            )
                        )
                        if k < len(interleave):
                            interleave[k]()
                            k += 1
                for h in (2 * hp, 2 * hp + 1):
                    ordered_act(
                        nc.scalar.activation(
                            mu[h], mu[h], AF.Ln, bias=1.0
                        )
                    )
                    # per-head preprocessing (overlaps later QK/V matmuls):
                    # qt rows 0:64 = mu_q cos(tw)/8, 64:128 = mu_q sin(tw)/8
                    # kt rows 0:64 = mu_k cos(tw+d), 64:128 = mu_k sin(tw+d)
                    # partition swap via SBUF->SBUF DMA on the idle SP queue
                    # (vs 2194ns of precious DVE per stream_shuffle), halved
                    # and interleaved so the first S matmuls start ~2us
                    # after the ln instead of ~5us
                    mks = shufp.tile([128, T], BF16, tag="mks", name="mks")
                    for th in range(2):
                        ts_ = slice(th * 1024, th * 1024 + 1024)
                        nc.sync.dma_start(mks[0:64, ts_], mu[h][64:128, ts_])
                        nc.sync.dma_start(mks[64:128, ts_], mu[h][0:64, ts_])
                        # mks-independent halves first (no DMA wait)
                        nc.gpsimd.tensor_mul(
                            qt[h][0:64, ts_], mu[h][0:64, ts_], trig[0:64, ts_]
                        )
                        nc.vector.tensor_mul(
                            kt[h][64:128, ts_], mu[h][64:128, ts_],
                            ab[64:128, h, ts_],
                        )
                        nc.vector.tensor_mul(
                            kt[h][0:64, ts_], mks[0:64, ts_], ab[0:64, h, ts_]
                        )
                        nc.gpsimd.tensor_mul(
                            qt[h][64:128, ts_], mks[64:128, ts_],
                            trig[64:128, ts_],
                        )

            # Phase A pairs with the j0 V chunks as PE filler while the
            # first pair's lns + preprocessing drain on ACT/DVE
            phaseA_pair(0)
            v_chunk(range(0, 4))

            # ------------- Phase B: attention + fused c_proj -------------
            HPCR = (0, 1, 2, 3)
            ypn_ref = [None]

            def c_proj_tt(tt, last=False):
                for eh in range(2):
                    po = ps_o.tile([128, 512], F32, tag="o", name="ps_o")
                    for ct in range(2):
                        nc.tensor.matmul(
                            po,
                            lhsT=y_t[:, ct, tt * 128 : (tt + 1) * 128],
                            rhs=w2[:, ct, eh * 512 : eh * 512 + 512],
                            start=(ct == 0),
                            stop=(ct == 1),
                        )
                    ost = ostp.tile([128, 512], F32, tag="ost", name="ost")
                    if last and eh == 1:
                        # final chunk: ACT is drained by now - use it for
                        # the copy (Copy is in every act table) + DMA so
                        # the tail isn't serialized on SP/Pool
                        nc.scalar.activation(ost, po, AF.Copy)
                        nc.scalar.dma_start(
                            out_d[tt][:, eh * 512 : eh * 512 + 512], ost
                        )
                    elif eh == 0:
                        nc.vector.tensor_copy(out=ost, in_=po)
                        nc.sync.dma_start(
                            out_d[tt][:, eh * 512 : eh * 512 + 512], ost
                        )
                    else:
                        nc.vector.tensor_copy(out=ost, in_=po)
                        nc.gpsimd.dma_start(
                            out_d[tt][:, eh * 512 : eh * 512 + 512], ost
                        )

            def flush_stages(pend):
                """Stages of the y-sweep + normalize (+transpose/c_proj)
                for a completed (j, h) score pass, to be interleaved one
                per S-pair of the next pass so the PE queue never sees a
                long non-score burst. b-outer keeps at most one pending
                psum accumulation group per bank."""
                jf, hf, p_map = pend
                yp = ps_y.tile([128, 4, 65], F32, tag="y", name="ps_y")

                def sweep(b_):
                    for i in range(4 * jf + b_ + 1):
                        tile_, sub = p_map[i]
                        nc.tensor.matmul(
                            yp[:, b_, :],
                            lhsT=tile_[
                                :, sub * 512 + b_ * 128 : sub * 512 + b_ * 128 + 128
                            ],
                            rhs=v_aug[:, i, hf, :],
                            start=(i == 0),
                            stop=(i == 4 * jf + b_),
                        )

                def normalize():
                    rc = rcp.tile([128, 4], F32, tag="rc", name="rc")
                    with nc.allow_low_precision(
                        reason="f32 reciprocal of softmax denominator"
                    ):
                        nc.vector.reciprocal(rc, yp[:, :, 64])
                    if hf % 2 == 0:
                        ypn_ref[0] = ynp.tile(
                            [128, 4, 2, 64], BF16, tag="yn", name="ypn"
                        )
                    ypn = ypn_ref[0]
                    for b_ in range(4):
                        nc.vector.tensor_scalar_mul(
                            ypn[:, b_, hf % 2, :], yp[:, b_, 0:64], rc[:, b_ : b_ + 1]
                        )

                def finalize():
                    if hf % 2 == 0:
                        return
                    # transpose 2 heads x [tq-128, 64] -> [128, tq-128]
                    hp = hf // 2
                    ypn = ypn_ref[0]
                    tp = ps_t.tile([128, 512], BF16, tag="t", name="ps_t")
                    for b_ in range(4):
                        nc.tensor.transpose(
                            tp[:, b_ * 128 : (b_ + 1) * 128],
                            ypn[:, b_].rearrange("p a d -> p (a d)"),
                            ident,
                        )
                    if hf == 3:
                        # per-b copy so c_proj(tt) streams out early
                        for b_ in range(4):
                            nc.vector.tensor_copy(
                                out=y_t[
                                    :, hp, jf * 512 + b_ * 128 : jf * 512 + b_ * 128 + 128
                                ],
                                in_=tp[:, b_ * 128 : (b_ + 1) * 128],
                            )
                            c_proj_tt(4 * jf + b_, last=(jf == 0))
                    else:
                        nc.vector.tensor_copy(
                            out=y_t[:, hp, jf * 512 : (jf + 1) * 512], in_=tp
                        )

                if jf == 0 and hf == 3:
                    # very last pass: per-b chains, largest b first, so
                    # the kernel tail is only b=0's short chain
                    def tail_b(b_):
                        sweep(b_)
                        rc = rcp.tile([128, 1], F32, tag="rc1", name="rc1")
                        with nc.allow_low_precision(
                            reason="f32 reciprocal of softmax denominator"
                        ):
                            nc.vector.reciprocal(rc, yp[:, b_, 64:65])
                        ypn = ypn_ref[0]
                        nc.vector.tensor_scalar_mul(
                            ypn[:, b_, 1, :], yp[:, b_, 0:64], rc
                        )
                        tp = ps_t.tile([128, 512], BF16, tag="t", name="ps_t")
                        nc.tensor.transpose(
                            tp[:, 0:128],
                            ypn[:, b_].rearrange("p a d -> p (a d)"), ident,
                        )
                        nc.vector.tensor_copy(
                            out=y_t[:, 1, b_ * 128 : b_ * 128 + 128],
                            in_=tp[:, 0:128],
                        )
                        c_proj_tt(b_, last=True)

                    return [
                        lambda: tail_b(3),
                        lambda: tail_b(2),
                        lambda: tail_b(1),
                        lambda: tail_b(0),
                    ]
                return [
                    lambda: sweep(0),
                    lambda: sweep(1),
                    lambda: sweep(2),
                    lambda: (sweep(3), normalize()),
                    finalize,
                ]

            pending = []

            def block_pairs(j):
                # pair each diagonal block (as sub0, so its valid range
                # [c0:512] abuts sub1) with a non-diagonal block: the exp
                # range [c0:1024] stays contiguous -> one ACT op per tile
                nblk = 4 * j + 4
                diag = list(range(4 * j, nblk))
                nond = list(range(0, 4 * j))
                if not nond:
                    return [(0, 1), (2, 3)]
                pairs = list(zip(diag, nond[: len(diag)]))
                rest = nond[len(diag) :]
                return pairs + [(rest[k], rest[k + 1]) for k in range(0, len(rest), 2)]

            def emit_score_pair(j, h, i0, i1, p_map, chained):
                sp = ps_s.tile([128, 1024], F32, tag="s", name="ps_s")
                p_sb = pp.tile([128, 1024], BF16, tag="p", name="p_sb")
                for sub, i in ((0, i0), (1, i1)):
                    c0 = 128 * max(0, i - 4 * j)
                    nc.tensor.matmul(
                        sp[:, sub * 512 + c0 : sub * 512 + 512],
                        lhsT=kt[h][:, i * 128 : (i + 1) * 128],
                        rhs=qt[h][:, j * 512 + c0 : (j + 1) * 512],
                        start=True,
                        stop=True,
                    )
                    p_map[i] = (p_sb, sub)
                c00 = 128 * max(0, i0 - 4 * j)
                c01 = 128 * max(0, i1 - 4 * j)
                if c01 == 0:
                    e = nc.scalar.activation(
                        p_sb[:, c00:1024], sp[:, c00:1024], AF.Exp, bias=bias_m1
                    )
                else:  # j=0: both blocks diagonal, 2 ops
                    nc.scalar.activation(
                        p_sb[:, c00:512], sp[:, c00:512], AF.Exp, bias=bias_m1
                    )
                    e = nc.scalar.activation(
                        p_sb[:, 512 + c01 : 1024],
                        sp[:, 512 + c01 : 1024],
                        AF.Exp,
                        bias=bias_m1,
                    )
                if chained:
                    # keep this exp in the Phase A ACT chain so the
                    # scheduler can't interleave it with an Ln
                    ordered_act(e)
                # mask diagonal strips (Pool)
                for sub, i in ((0, i0), (1, i1)):
                    if 4 * j <= i <= 4 * j + 3:
                        boff = sub * 512 + 128 * (i - 4 * j)
                        nc.gpsimd.tensor_mul(
                            p_sb[:, boff : boff + 128],
                            p_sb[:, boff : boff + 128],
                            cmask,
                        )

            def emit_score_block(j, h, i, p_map):
                # single tk-block score tile in the ps_o pool: used to
                # interleave (j1,h0) into the QK phase without touching
                # the ps_s rotation that paces the QK psums
                sp = ps_o.tile([128, 512], F32, tag="o", name="ps_o")
                p_sb = pp.tile([128, 512], BF16, tag="p1", name="p_sb1")
                c0 = 128 * max(0, i - 4 * j)
                nc.tensor.matmul(
                    sp[:, c0:512],
                    lhsT=kt[h][:, i * 128 : (i + 1) * 128],
                    rhs=qt[h][:, j * 512 + c0 : (j + 1) * 512],
                    start=True,
                    stop=True,
                )
                p_map[i] = (p_sb, 0)
                ordered_act(
                    nc.scalar.activation(
                        p_sb[:, c0:512], sp[:, c0:512], AF.Exp, bias=bias_m1
                    )
                )
                if 4 * j <= i <= 4 * j + 3:
                    boff = 128 * (i - 4 * j)
                    nc.gpsimd.tensor_mul(
                        p_sb[:, boff : boff + 128],
                        p_sb[:, boff : boff + 128],
                        cmask,
                    )

            def score_pass(j, h):
                p_map = {}
                for i0, i1 in block_pairs(j):
                    emit_score_pair(j, h, i0, i1, p_map, chained=False)
                    # consume one flush stage of the previous (j, h)
                    # behind each S pair: the PE stream alternates score
                    # matmuls with y-sweep chunks and ACT stays fed
                    if pending:
                        pending.pop(0)()
                # carry at most the finalize stage into the next pass (the
                # P tiles are released once sweep3 ran, keeping pp bounded)
                while len(pending) > 3:
                    pending.pop(0)()
                pending.extend(flush_stages((j, h, p_map)))

            # (j1, h0) rides inside pair 1: its single-block score tiles
            # live in ps_o (so the QK psum rotation is untouched) and its
            # exps chain between the QK exps, filling the ACT idle of the
            # PE-bound QK stretch
            p_map10 = {}
            dist = [(0, 1, 2), (3, 4, 5), (6,), (7,)]
            phaseA_pair(1, interleave=[
                (lambda blks=bl: [emit_score_block(1, 0, i, p_map10)
                                  for i in blks])
                for bl in dist
            ])
            pending.extend(flush_stages((1, 0, p_map10)))
            v_chunk(range(4, 8))
            for j, hs in ((1, (1, 2, 3)), (2, HPCR), (3, HPCR), (0, HPCR)):
                if j in (2, 3):
                    v_chunk(range(4 * j, 4 * j + 4))
                for h in hs:
                    score_pass(j, h)
            while pending:
                pending.pop(0)()

    nc.compile()
    return nc


def make_inputs(x, w_attn, w_proj, delta):
    """Host-side prep: per-core input dicts (core = b*4 + g)."""
    x = np.asarray(x, dtype=np.float32)
    w_attn = np.asarray(w_attn, dtype=np.float32)
    w_proj = np.asarray(w_proj, dtype=np.float32)
    delta = np.asarray(delta, dtype=np.float32)
    bf = ml_dtypes.bfloat16

    inv_freq = 1.0 / (BASE ** (np.arange(D, dtype=np.float32) / D))
    t = np.arange(T, dtype=np.float32)
    freqs = t[:, None] * inv_freq[None, :]  # (T, D)
    scale = 1.0 / math.sqrt(D)
    trig = np.concatenate(
        [np.cos(freqs).T * scale, np.sin(freqs).T * scale], axis=0
    ).astype(bf)  # (128, T)

    d = np.clip(delta, -2.0 * math.pi, 0.0)

    qw = w_attn[:C].reshape(H, D, C)
    kw = w_attn[C : 2 * C].reshape(H, D, C)
    vw = w_attn[2 * C :].reshape(H, D, C)

    # causal mask for diagonal 128-blocks of P^T [tk, tq]: valid iff tq >= tk
    tk = np.arange(128)[:, None]
    cc = np.arange(128)[None, :]
    cmask = (cc >= tk).astype(bf)
    ident = np.eye(128, dtype=np.float32).astype(bf)

    in_maps = []
    for core in range(N_CORES):
        b, g = divmod(core, HPC)
        heads = range(HPC * g, HPC * g + HPC)

        xT = np.ascontiguousarray(x[b].T).reshape(NCT, 128, T).astype(bf)

        qk = np.stack(
            [np.concatenate([qw[h], kw[h]], axis=0) for h in heads], axis=0
        )  # (4, 128, C)
        wqk = np.ascontiguousarray(qk.transpose(2, 0, 1).reshape(C, 512)).reshape(
            NCT, 128, 512
        ).astype(bf)
        wv = np.ascontiguousarray(
            vw[HPC * g : HPC * g + HPC].reshape(256, C).T
        ).reshape(NCT, 128, 256).astype(bf)
        w2t = np.ascontiguousarray(
            w_proj[:, 256 * g : 256 * (g + 1)].T
        ).reshape(2, 128, 1024).astype(bf)

        ab = np.stack(
            [
                np.concatenate(
                    [
                        np.cos(freqs + d[h][None, :]).T,
                        np.sin(freqs + d[h][None, :]).T,
                    ],
                    axis=0,
                )
                for h in heads
            ],
            axis=0,
        ).astype(bf)  # (4, 128, T)

        in_maps.append(
            {
                "xT": xT,
                "wqk": wqk,
                "wv": wv,
                "w2": w2t,
                "trig": trig,
                "ab": ab,
                "cmask": cmask,
                "ident": ident,
            }
        )
    return in_maps


_NC_CACHE = []


def _get_nc():
    if not _NC_CACHE:
        _NC_CACHE.append(build_module())
    return _NC_CACHE[0]


def kernel(x, w_attn, w_proj, delta, _trace=False):
    in_maps = make_inputs(x, w_attn, w_proj, delta)
    nc = _get_nc()
    res = None
    outs = None
    last_err = None
    for attempt in range(3):
        try:
            res = bass_utils.run_bass_kernel_spmd(
                nc, in_maps, core_ids=list(range(N_CORES)), trace=_trace
            )
            outs = [
                np.asarray(r["out"]).reshape(T, C) for r in res.results
            ]
            break
        except Exception as e:
            last_err = e
            if "unrecoverable" not in str(e).lower() or attempt == 2:
                raise
            import time as _time

            _time.sleep(2.0)
    assert outs is not None, last_err
    if _trace:
        kernel.last_results = res
    full = np.zeros((B, T, C), dtype=np.float32)
    for core in range(N_CORES):
        full[core // HPC] += outs[core]
    return full


# revision 80
# speedup vs baseline: 1.0018x; 1.0018x over previous
"""Trainium2 Bass kernel for causal self-attention with PoPE (v2).

Reference computation (B=2, T=2048, C=1024, H=16, D=64):
  qkv = x @ w_attn.T ; split q,k,v ; heads
  mu_q = softplus(q); mu_k = softplus(k)
  q_real = mu_q * cos(t w); q_imag = mu_q * sin(t w)
  k_real = mu_k * cos(t w + d); k_imag = mu_k * sin(t w + d)
  att = softmax_causal((q_real k_real + q_imag k_imag)/sqrt(D))
  y = att @ v ; out = y @ w_proj.T

Sharding: 8 cores = 2 batches x 4 head-groups (4 heads each). Each core
computes its batch's QKV for its heads, attention, and a partial c_proj
(its heads' input-channel rows of w_proj). Host sums the 4 partials per
batch.

v2 design (132.9us vs the previous 172.7us version; ACT ~110us busy
is the bottleneck engine, PE ~100us):
  * all-bf16 tensors (the NEFF verifier rejects mixed f32r/bf16
    matmuls): halves DMA bytes, keeps every matmul at 1 cycle/row, and
    doubles DVE throughput on the elementwise path.
  * y matmul flipped to out[tq,d] (free dim 65 instead of 512): halves
    its PE cost AND lands the softmax denominator as a per-partition
    scalar (psum col 64), so normalization is a [128,4] reciprocal + 4
    tensor_scalar ops instead of reciprocal+PE-broadcast+mul. The sweep
    runs b-outer so only one psum accumulation group is open per bank
    (ZERO_REGION = a full 2KB bank).
  * y^T for c_proj via PE transpose (2 heads x 1 tq-128 block per op;
    gpsimd cannot read PSUM, so all psum->sbuf copies ride DVE/ACT).
  * c_proj fused per tq-128 block right after the last head's
    transpose: output DMAs spread across the whole attention phase; the
    very last pass runs per-b chains, largest b first, so the kernel
    tail is one short chain.
  * Phase A softplus = exp+ln batched per head-pair with explicit ACT
    ordering (the act-table pass otherwise reloads tables on every
    Exp<->Ln switch: 5 loads instead of 15); diagonal score blocks pair
    with non-diagonal ones so each psum tile needs a single contiguous
    exp.
  * every psum pool is opened once at the top (QK shares the S pool's
    tag) - no pool-boundary chaining stalls at phase transitions.
  * the (j,h) y-sweep is split into stages consumed one per S-pair of
    the following passes (up to 3 carried), so the in-order PE queue
    alternates score matmuls with sweep chunks and ACT never starves.
  * mu partition swaps via SBUF->SBUF DMA on the idle SP queue instead
    of DVE stream_shuffle; xT DMAs split per tbp-half across all three
    DMA-capable queues so the first QK psum completes ~6us in.
"""

import math
import os
import sys

import numpy as np

for _p in ("/opt/trn_rl_repo",):
    if _p not in sys.path and os.path.isdir(_p):
        sys.path.insert(0, _p)

import ml_dtypes

import concourse.tile as tile
from concourse import bacc
from concourse import mybir
from concourse import bass_utils

B, T, C = 2, 2048, 1024
H, D = 16, 64
BASE = 10000.0
N_CORES = 8
HPC = 4  # heads per core
NCT = 8  # c tiles (1024/128)
NTT = 16  # t tiles of 128

F32 = mybir.dt.float32
F32R = mybir.dt.float32r
BF16 = mybir.dt.bfloat16
AF = mybir.ActivationFunctionType


def build_module():
    nc = bacc.Bacc(
        "TRN2", target_bir_lowering=False, debug=False, num_devices=N_CORES
    )

    xT_d = nc.dram_tensor("xT", (NCT, 128, T), BF16, kind="ExternalInput").ap()
    wqk_d = nc.dram_tensor("wqk", (NCT, 128, 512), BF16, kind="ExternalInput").ap()
    wv_d = nc.dram_tensor("wv", (NCT, 128, 256), BF16, kind="ExternalInput").ap()
    w2_d = nc.dram_tensor("w2", (2, 128, 1024), BF16, kind="ExternalInput").ap()
    trig_d = nc.dram_tensor("trig", (128, T), BF16, kind="ExternalInput").ap()
    ab_d = nc.dram_tensor("ab", (HPC, 128, T), BF16, kind="ExternalInput").ap()
    cmask_d = nc.dram_tensor("cmask", (128, 128), BF16, kind="ExternalInput").ap()
    ident_d = nc.dram_tensor("ident", (128, 128), BF16, kind="ExternalInput").ap()
    out_d = nc.dram_tensor("out", (NTT, 128, 1024), F32, kind="ExternalOutput").ap()

    idm = list(range(32))

    with tile.TileContext(nc) as tc:
        with (
            tc.tile_pool(name="persist", bufs=1) as persist,
            tc.tile_pool(name="shufp", bufs=3) as shufp,
            tc.tile_pool(name="pp", bufs=12) as pp,
            tc.tile_pool(name="rcp", bufs=4) as rcp,
            tc.tile_pool(name="ynp", bufs=3) as ynp,
            tc.tile_pool(name="ostp", bufs=8) as ostp,
            tc.tile_pool(name="ps_s", bufs=2, space="PSUM") as ps_s,
            tc.tile_pool(name="ps_y", bufs=1, space="PSUM") as ps_y,
            tc.tile_pool(name="ps_t", bufs=1, space="PSUM") as ps_t,
            tc.tile_pool(name="ps_o", bufs=2, space="PSUM") as ps_o,
        ):
            # ---------------- persistent tiles ----------------
            v_aug = persist.tile([128, NTT, HPC, 65], BF16)
            nc.vector.memset(v_aug[:, :, :, 64:65], 1.0)
            trig = persist.tile([128, T], BF16)
            ab = persist.tile([128, HPC, T], BF16)
            cmask = persist.tile([128, 128], BF16)
            ident = persist.tile([128, 128], BF16)
            bias_m1 = persist.tile([128, 1], F32)
            nc.vector.memset(bias_m1, -1.0)
            mu = [persist.tile([128, T], BF16, name=f"mu{h}") for h in range(HPC)]
            qt = [persist.tile([128, T], BF16, name=f"qt{h}") for h in range(HPC)]
            kt = [persist.tile([128, T], BF16, name=f"kt{h}") for h in range(HPC)]
            y_t = persist.tile([128, 2, T], BF16)
            w2 = persist.tile([128, 2, 1024], BF16)
            wqk = persist.tile([128, NCT, 512], BF16)
            wv = persist.tile([128, NCT, 256], BF16)
            xT = persist.tile([128, NCT, T], BF16)

            # ------------- input DMAs (chunked; ACT only at idle start) ----
            xt_eng = {0: nc.sync, 3: nc.sync, 6: nc.sync,
                      1: nc.gpsimd, 4: nc.gpsimd, 7: nc.gpsimd,
                      2: nc.scalar, 5: nc.scalar}
            for c in range(NCT):
                weng = nc.gpsimd if c >= 6 else nc.sync
                weng.dma_start(wqk[:, c, :], wqk_d[c])
                # tbp-half granularity: the first QK psum only needs the
                # t<1024 halves, so its matmuls start ~2us earlier
                xt_eng[c].dma_start(xT[:, c, 0:1024], xT_d[c][:, 0:1024])
            for c in range(NCT):
                xt_eng[c].dma_start(xT[:, c, 1024:2048], xT_d[c][:, 1024:2048])
            nc.gpsimd.dma_start(wv, wv_d.rearrange("o p e -> p o e"))
            # trig/ab0-1 on the ACT queue: it is idle until the first QK
            # psum completes, so these transfers are free there; ab2-3
            # ride late on SP (needed only at prep h2/h3)
            nc.scalar.dma_start(trig, trig_d)
            for h in range(2):
                nc.scalar.dma_start(ab[:, h, :], ab_d[h])
            nc.gpsimd.dma_start(cmask, cmask_d)
            nc.sync.dma_start(ident, ident_d)
            nc.sync.dma_start(w2, w2_d.rearrange("o p e -> p o e"))
            for h in range(2, HPC):
                nc.sync.dma_start(ab[:, h, :], ab_d[h])

            # ---------------- Phase A: QKV projection ----------------
            # exp/ln batched per head-pair, with explicit ACT ordering so
            # the scheduler can't interleave Exp<->Ln (each switch costs a
            # 1283ns act-table reload): 5 loads total instead of 15.
            from concourse.tile_rust import add_dep_helper

            last_act = [None]

            def ordered_act(handle):
                if last_act[0] is not None:
                    add_dep_helper(
                        handle.ins, last_act[0].ins, sync=False,
                        reason="batch ACT ops to minimize table reloads",
                    )
                last_act[0] = handle

            def v_chunk(tts):
                # V tiles borrow the ps_o pool (temporally disjoint from
                # c_proj use: V at chunk start, c_proj at end)
                for tt in tts:
                    psv = ps_o.tile([128, 512], F32, tag="o", name="ps_o")
                    for c in range(NCT):
                        nc.tensor.matmul(
                            psv[:, 0:256],
                            lhsT=xT[:, c, tt * 128 : (tt + 1) * 128],
                            rhs=wv[:, c, :],
                            start=(c == 0),
                            stop=(c == NCT - 1),
                        )
                    nc.vector.tensor_copy(
                        out=v_aug[:, tt, :, 0:64],
                        in_=psv[:, 0:256].rearrange("p (h e) -> p h e", h=HPC),
                    )

            def phaseA_pair(hp, interleave=()):
                k = 0
                for h in (2 * hp, 2 * hp + 1):
                    for tbp in range(2):
                        ps = ps_s.tile([128, 1024], F32, tag="s", name="ps_s")
                        base = tbp * 1024
                        for c in range(NCT):
                            for half in range(2):
                                o0 = half * 512
                                nc.tensor.matmul(
                                    ps[:, o0 : o0 + 512],
                                    lhsT=wqk[:, c, h * 128 : (h + 1) * 128],
                                    rhs=xT[:, c, base + o0 : base + o0 + 512],
                                    start=(c == 0),
                                    stop=(c == NCT - 1),
                                )
                        ordered_act(
                            nc.scalar.activation(
                                mu[h][:, base : base + 1024], ps, AF.Exp
                            )
                        )
                        if k < len(interleave):
                            interleave[k]()
                            k += 1
                for h in (2 * hp, 2 * hp + 1):
                    ordered_act(
                        nc.scalar.activation(
                            mu[h], mu[h], AF.Ln, bias=1.0
                        )
                    )
                    # per-head preprocessing (overlaps later QK/V matmuls):
                    # qt rows 0:64 = mu_q cos(tw)/8, 64:128 = mu_q sin(tw)/8
                    # kt rows 0:64 = mu_k cos(tw+d), 64:128 = mu_k sin(tw+d)
                    # partition swap via SBUF->SBUF DMA on the idle SP queue
                    # (vs 2194ns of precious DVE per stream_shuffle), halved
                    # and interleaved so the first S matmuls start ~2us
                    # after the ln instead of ~5us
                    mks = shufp.tile([128, T], BF16, tag="mks", name="mks")
                    for th in range(2):
                        ts_ = slice(th * 1024, th * 1024 + 1024)
                        nc.sync.dma_start(mks[0:64, ts_], mu[h][64:128, ts_])
                        nc.sync.dma_start(mks[64:128, ts_], mu[h][0:64, ts_])
                        # mks-independent halves first (no DMA wait)
                        nc.gpsimd.tensor_mul(
                            qt[h][0:64, ts_], mu[h][0:64, ts_], trig[0:64, ts_]
                        )
                        nc.vector.tensor_mul(
                            kt[h][64:128, ts_], mu[h][64:128, ts_],
                            ab[64:128, h, ts_],
                        )
                        nc.vector.tensor_mul(
                            kt[h][0:64, ts_], mks[0:64, ts_], ab[0:64, h, ts_]
                        )
                        nc.gpsimd.tensor_mul(
                            qt[h][64:128, ts_], mks[64:128, ts_],
                            trig[64:128, ts_],
                        )

            # Phase A pairs with the j0 V chunks as PE filler while the
            # first pair's lns + preprocessing drain on ACT/DVE
            phaseA_pair(0)
            v_chunk(range(0, 4))

            # ------------- Phase B: attention + fused c_proj -------------
            HPCR = (0, 1, 2, 3)
            ypn_ref = [None]

            def c_proj_tt(tt, last=False):
                for eh in range(2):
                    po = ps_o.tile([128, 512], F32, tag="o", name="ps_o")
                    for ct in range(2):
                        nc.tensor.matmul(
                            po,
                            lhsT=y_t[:, ct, tt * 128 : (tt + 1) * 128],
                            rhs=w2[:, ct, eh * 512 : eh * 512 + 512],
                            start=(ct == 0),
                            stop=(ct == 1),
                        )
                    ost = ostp.tile([128, 512], F32, tag="ost", name="ost")
                    if last and eh == 1:
                        # final chunk: ACT is drained by now - use it for
                        # the copy (Copy is in every act table) + DMA so
                        # the tail isn't serialized on SP/Pool
                        nc.scalar.activation(ost, po, AF.Copy)
                        nc.scalar.dma_start(
                            out_d[tt][:, eh * 512 : eh * 512 + 512], ost
                        )
                    elif eh == 0:
                        nc.vector.tensor_copy(out=ost, in_=po)
                        nc.sync.dma_start(
                            out_d[tt][:, eh * 512 : eh * 512 + 512], ost
                        )
                    else:
                        nc.vector.tensor_copy(out=ost, in_=po)
                        nc.gpsimd.dma_start(
                            out_d[tt][:, eh * 512 : eh * 512 + 512], ost
                        )

            def flush_stages(pend):
                """Stages of the y-sweep + normalize (+transpose/c_proj)
                for a completed (j, h) score pass, to be interleaved one
                per S-pair of the next pass so the PE queue never sees a
                long non-score burst. b-outer keeps at most one pending
                psum accumulation group per bank."""
                jf, hf, p_map = pend
                yp = ps_y.tile([128, 4, 65], F32, tag="y", name="ps_y")

                def sweep(b_):
                    for i in range(4 * jf + b_ + 1):
                        tile_, sub = p_map[i]
                        nc.tensor.matmul(
                            yp[:, b_, :],
                            lhsT=tile_[
                                :, sub * 512 + b_ * 128 : sub * 512 + b_ * 128 + 128
                            ],
                            rhs=v_aug[:, i, hf, :],
                            start=(i == 0),
                            stop=(i == 4 * jf + b_),
                        )

                def normalize():
                    rc = rcp.tile([128, 4], F32, tag="rc", name="rc")
                    with nc.allow_low_precision(
                        reason="f32 reciprocal of softmax denominator"
                    ):
                        nc.vector.reciprocal(rc, yp[:, :, 64])
                    if hf % 2 == 0:
                        ypn_ref[0] = ynp.tile(
                            [128, 4, 2, 64], BF16, tag="yn", name="ypn"
                        )
                    ypn = ypn_ref[0]
                    for b_ in range(4):
                        nc.vector.tensor_scalar_mul(
                            ypn[:, b_, hf % 2, :], yp[:, b_, 0:64], rc[:, b_ : b_ + 1]
                        )

                def finalize():
                    if hf % 2 == 0:
                        return
                    # transpose 2 heads x [tq-128, 64] -> [128, tq-128]
                    hp = hf // 2
                    ypn = ypn_ref[0]
                    tp = ps_t.tile([128, 512], BF16, tag="t", name="ps_t")
                    for b_ in range(4):
                        nc.tensor.transpose(
                            tp[:, b_ * 128 : (b_ + 1) * 128],
                            ypn[:, b_].rearrange("p a d -> p (a d)"),
                            ident,
                        )
                    if hf == 3:
                        # per-b copy so c_proj(tt) streams out early
                        for b_ in range(4):
                            nc.vector.tensor_copy(
                                out=y_t[
                                    :, hp, jf * 512 + b_ * 128 : jf * 512 + b_ * 128 + 128
                                ],
                                in_=tp[:, b_ * 128 : (b_ + 1) * 128],
                            )
                            c_proj_tt(4 * jf + b_, last=(jf == 0))
                    else:
                        nc.vector.tensor_copy(
                            out=y_t[:, hp, jf * 512 : (jf + 1) * 512], in_=tp
                        )

                if jf == 0 and hf == 3:
                    # very last pass: per-b chains, largest b first, so
                    # the kernel tail is only b=0's short chain
                    def tail_b(b_):
                        sweep(b_)
                        rc = rcp.tile([128, 1], F32, tag="rc1", name="rc1")
                        with nc.allow_low_precision(
                            reason="f32 reciprocal of softmax denominator"
                        ):
                            nc.vector.reciprocal(rc, yp[:, b_, 64:65])
                        ypn = ypn_ref[0]
                        nc.vector.tensor_scalar_mul(
                            ypn[:, b_, 1, :], yp[:, b_, 0:64], rc
                        )
                        tp = ps_t.tile([128, 512], BF16, tag="t", name="ps_t")
                        nc.tensor.transpose(
                            tp[:, 0:128],
                            ypn[:, b_].rearrange("p a d -> p (a d)"), ident,
                        )
                        nc.vector.tensor_copy(
                            out=y_t[:, 1, b_ * 128 : b_ * 128 + 128],
                            in_=tp[:, 0:128],
                        )
                        c_proj_tt(b_, last=True)

                    return [
                        lambda: tail_b(3),
                        lambda: tail_b(2),
                        lambda: tail_b(1),
                        lambda: tail_b(0),
                    ]
                return [
                    lambda: sweep(0),
                    lambda: sweep(1),
                    lambda: sweep(2),
                    lambda: (sweep(3), normalize()),
                    finalize,
                ]

            pending = []

            def block_pairs(j):
                # pair each diagonal block (as sub0, so its valid range
                # [c0:512] abuts sub1) with a non-diagonal block: the exp
                # range [c0:1024] stays contiguous -> one ACT op per tile
                nblk = 4 * j + 4
                diag = list(range(4 * j, nblk))
                nond = list(range(0, 4 * j))
                if not nond:
                    return [(0, 1), (2, 3)]
                pairs = list(zip(diag, nond[: len(diag)]))
                rest = nond[len(diag) :]
                return pairs + [(rest[k], rest[k + 1]) for k in range(0, len(rest), 2)]

            def emit_score_pair(j, h, i0, i1, p_map, chained):
                sp = ps_s.tile([128, 1024], F32, tag="s", name="ps_s")
                p_sb = pp.tile([128, 1024], BF16, tag="p", name="p_sb")
                for sub, i in ((0, i0), (1, i1)):
                    c0 = 128 * max(0, i - 4 * j)
                    nc.tensor.matmul(
                        sp[:, sub * 512 + c0 : sub * 512 + 512],
                        lhsT=kt[h][:, i * 128 : (i + 1) * 128],
                        rhs=qt[h][:, j * 512 + c0 : (j + 1) * 512],
                        start=True,
                        stop=True,
                    )
                    p_map[i] = (p_sb, sub)
                c00 = 128 * max(0, i0 - 4 * j)
                c01 = 128 * max(0, i1 - 4 * j)
                if c01 == 0:
                    e = nc.scalar.activation(
                        p_sb[:, c00:1024], sp[:, c00:1024], AF.Exp, bias=bias_m1
                    )
                else:  # j=0: both blocks diagonal, 2 ops
                    nc.scalar.activation(
                        p_sb[:, c00:512], sp[:, c00:512], AF.Exp, bias=bias_m1
                    )
                    e = nc.scalar.activation(
                        p_sb[:, 512 + c01 : 1024],
                        sp[:, 512 + c01 : 1024],
                        AF.Exp,
                        bias=bias_m1,
                    )
                if chained:
                    # keep this exp in the Phase A ACT chain so the
                    # scheduler can't interleave it with an Ln
                    ordered_act(e)
                # mask diagonal strips (Pool)
                for sub, i in ((0, i0), (1, i1)):
                    if 4 * j <= i <= 4 * j + 3:
                        boff = sub * 512 + 128 * (i - 4 * j)
                        nc.gpsimd.tensor_mul(
                            p_sb[:, boff : boff + 128],
                            p_sb[:, boff : boff + 128],
                            cmask,
                        )

            def emit_score_block(j, h, i, p_map):
                # single tk-block score tile in the ps_o pool: used to
                # interleave (j1,h0) into the QK phase without touching
                # the ps_s rotation that paces the QK psums
                sp = ps_o.tile([128, 512], F32, tag="o", name="ps_o")
                p_sb = pp.tile([128, 512], BF16, tag="p1", name="p_sb1")
                c0 = 128 * max(0, i - 4 * j)
                nc.tensor.matmul(
                    sp[:, c0:512],
                    lhsT=kt[h][:, i * 128 : (i + 1) * 128],
                    rhs=qt[h][:, j * 512 + c0 : (j + 1) * 512],
                    start=True,
                    stop=True,
                )
                p_map[i] = (p_sb, 0)
                ordered_act(
                    nc.scalar.activation(
                        p_sb[:, c0:512], sp[:, c0:512], AF.Exp, bias=bias_m1
                    )
                )
                if 4 * j <= i <= 4 * j + 3:
                    boff = 128 * (i - 4 * j)
                    nc.gpsimd.tensor_mul(
                        p_sb[:, boff : boff + 128],
                        p_sb[:, boff : boff + 128],
                        cmask,
                    )

            def score_pass(j, h):
                p_map = {}
                for i0, i1 in block_pairs(j):
                    emit_score_pair(j, h, i0, i1, p_map, chained=False)
                    # consume one flush stage of the previous (j, h)
                    # behind each S pair: the PE stream alternates score
                    # matmuls with y-sweep chunks and ACT stays fed
                    if pending:
                        pending.pop(0)()
                # carry at most the finalize stage into the next pass (the
                # P tiles are released once sweep3 ran, keeping pp bounded)
                while len(pending) > 3:
                    pending.pop(0)()
                pending.extend(flush_stages((j, h, p_map)))

            # (j1, h0) rides inside pair 1: its single-block score tiles
            # live in ps_o (so the QK psum rotation is untouched) and its
            # exps chain between the QK exps, filling the ACT idle of the
            # PE-bound QK stretch
            p_map10 = {}
            dist = [(0, 1, 2, 3, 4, 5), (6, 7), (), ()]
            phaseA_pair(1, interleave=[
                (lambda blks=bl: [emit_score_block(1, 0, i, p_map10)
                                  for i in blks])
                for bl in dist
            ])
            pending.extend(flush_stages((1, 0, p_map10)))
            v_chunk(range(4, 8))
            for j, hs in ((1, (1, 2, 3)), (2, HPCR), (3, HPCR), (0, HPCR)):
                if j in (2, 3):
                    v_chunk(range(4 * j, 4 * j + 4))
                for h in hs:
                    score_pass(j, h)
            while pending:
                pending.pop(0)()

    nc.compile()
    return nc


def make_inputs(x, w_attn, w_proj, delta):
    """Host-side prep: per-core input dicts (core = b*4 + g)."""
    x = np.asarray(x, dtype=np.float32)
    w_attn = np.asarray(w_attn, dtype=np.float32)
    w_proj = np.asarray(w_proj, dtype=np.float32)
    delta = np.asarray(delta, dtype=np.float32)
    bf = ml_dtypes.bfloat16

    inv_freq = 1.0 / (BASE ** (np.arange(D, dtype=np.float32) / D))
    t = np.arange(T, dtype=np.float32)
    freqs = t[:, None] * inv_freq[None, :]  # (T, D)
    scale = 1.0 / math.sqrt(D)
    trig = np.concatenate(
        [np.cos(freqs).T * scale, np.sin(freqs).T * scale], axis=0
    ).astype(bf)  # (128, T)

    d = np.clip(delta, -2.0 * math.pi, 0.0)

    qw = w_attn[:C].reshape(H, D, C)
    kw = w_attn[C : 2 * C].reshape(H, D, C)
    vw = w_attn[2 * C :].reshape(H, D, C)

    # causal mask for diagonal 128-blocks of P^T [tk, tq]: valid iff tq >= tk
    tk = np.arange(128)[:, None]
    cc = np.arange(128)[None, :]
    cmask = (cc >= tk).astype(bf)
    ident = np.eye(128, dtype=np.float32).astype(bf)

    in_maps = []
    for core in range(N_CORES):
        b, g = divmod(core, HPC)
        heads = range(HPC * g, HPC * g + HPC)

        xT = np.ascontiguousarray(x[b].T).reshape(NCT, 128, T).astype(bf)

        qk = np.stack(
            [np.concatenate([qw[h], kw[h]], axis=0) for h in heads], axis=0
        )  # (4, 128, C)
        wqk = np.ascontiguousarray(qk.transpose(2, 0, 1).reshape(C, 512)).reshape(
            NCT, 128, 512
        ).astype(bf)
        wv = np.ascontiguousarray(
            vw[HPC * g : HPC * g + HPC].reshape(256, C).T
        ).reshape(NCT, 128, 256).astype(bf)
        w2t = np.ascontiguousarray(
            w_proj[:, 256 * g : 256 * (g + 1)].T
        ).reshape(2, 128, 1024).astype(bf)

        ab = np.stack(
            [
                np.concatenate(
                    [
                        np.cos(freqs + d[h][None, :]).T,
                        np.sin(freqs + d[h][None, :]).T,
                    ],
                    axis=0,
                )
                for h in heads
            ],
            axis=0,
        ).astype(bf)  # (4, 128, T)

        in_maps.append(
            {
                "xT": xT,
                "wqk": wqk,
                "wv": wv,
                "w2": w2t,
                "trig": trig,
                "ab": ab,
                "cmask": cmask,
                "ident": ident,
            }
        )
    return in_maps


_NC_CACHE = []


def _get_nc():
    if not _NC_CACHE:
        _NC_CACHE.append(build_module())
    return _NC_CACHE[0]


def kernel(x, w_attn, w_proj, delta, _trace=False):
    in_maps = make_inputs(x, w_attn, w_proj, delta)
    nc = _get_nc()
    res = None
    outs = None
    last_err = None
    for attempt in range(3):
        try:
            res = bass_utils.run_bass_kernel_spmd(
                nc, in_maps, core_ids=list(range(N_CORES)), trace=_trace
            )
            outs = [
                np.asarray(r["out"]).reshape(T, C) for r in res.results
            ]
            break
        except Exception as e:
            last_err = e
            if "unrecoverable" not in str(e).lower() or attempt == 2:
                raise
            import time as _time

            _time.sleep(2.0)
    assert outs is not None, last_err
    if _trace:
        kernel.last_results = res
    full = np.zeros((B, T, C), dtype=np.float32)
    for core in range(N_CORES):
        full[core // HPC] += outs[core]
    return full
